# revision 28
# baseline (speedup 1.0000x reference)
"""KGAT forward kernel for 8 Trainium2 NeuronCores (Bass/Tile SPMD).

Strategy (dst-sharded graph parallel):
  - Nodes padded to NP = 8*PPC; core c owns rows [c*PPC, (c+1)*PPC).
  - Node tables T_l [NP, 64] f32 live replicated in each core's DRAM in an
    "image" layout: flat row index of node g = owner*PPC + (loc%128)*TPC + loc//128,
    so a core's piece maps 1:1 onto SBUF [128, TPC, 64] with node tile
    t = {128t+p} at column block t.
  - Edge phase per layer: per-edge source rows pulled with the custom
    dma_gather (int16 idx, windowed in 32768-row ranges), segment-sum by
    destination via one-hot matrices (iota is_equal) matmul-accumulated in
    PSUM per 128-destination tile.
  - Dense phase: X'=[X|1] per tile, PE transpose, bf16 matmul with
    W'=[W;b], leaky-relu on ACT, l2-norm factors kept (not applied) and
    packed into spare columns of T3 for use at scoring time.
  - AllGather (collective) replicates each new layer piece.
  - Scoring: batch shard per core; rows fetched with windowed gathers and
    re-ordered into batch order with unique-index dma_scatter_add into
    SBUF (parity-split); dot products on DVE.
"""
import sys
sys.path.insert(0, '/opt/trn_rl_repo')

import numpy as np
import ml_dtypes

import os
import concourse.bass as bass
import concourse.bacc as bacc
import concourse.tile as tile
from concourse import mybir
from concourse.bass_utils import run_bass_kernel_spmd

BF = ml_dtypes.bfloat16
NCORES = 8
WINDOW = 32768
ACT = mybir.ActivationFunctionType
ALU = mybir.AluOpType


def _wrap16(idx):
    """int16 idx array -> [128, n/16] wrapped+replicated layout."""
    n = len(idx)
    assert n % 16 == 0
    return np.tile(idx.reshape(n // 16, 16).T, (8, 1)).astype(np.int16)


def _img(loc, tpc):
    """local node id -> piece-flat image row."""
    return (loc % 128) * tpc + loc // 128


def build_host_data(inputs):
    """All host-side preprocessing. Returns (meta, in_maps)."""
    users = np.asarray(inputs["users"])
    pos_items = np.asarray(inputs["pos_items"])
    neg_items = np.asarray(inputs["neg_items"])
    rows = np.asarray(inputs["rows"]).astype(np.int64)
    cols = np.asarray(inputs["cols"]).astype(np.int64)
    vals = np.asarray(inputs["edge_vals"]).astype(np.float32)
    ue = np.asarray(inputs["user_embed"]).astype(np.float32)
    ee = np.asarray(inputs["entity_embed"]).astype(np.float32)

    NU, D0 = ue.shape
    NE = ee.shape[0]
    N = NU + NE
    B = users.shape[0]
    BPC = B // NCORES
    B3 = BPC // 128
    SC = 3 * B3

    PPC = -(-N // (NCORES * 128)) * 128
    NP = PPC * NCORES
    TPC = PPC // 128
    NRANGE = -(-NP // WINDOW)

    douts = [inputs[f"W_gc{l}"].shape[1] for l in range(3)]
    dins = [D0, douts[0], douts[1]]

    # --- node -> table flat row (image layout) ---
    def flat_of(g):
        c = g // PPC
        loc = g % PPC
        return c * PPC + _img(loc, TPC)

    # --- ego0 full table (image layout), fp32 ---
    ego0 = np.zeros((NP, 64), np.float32)
    allemb = np.concatenate([ue, ee], 0)
    ego0[flat_of(np.arange(N)), :D0] = allemb

    # --- per-core ego0 piece, SBUF image, bf16 ---
    ego0sb = []
    for c in range(NCORES):
        piece = ego0[c * PPC:(c + 1) * PPC]          # already image-ordered
        ego0sb.append(piece.reshape(128, TPC * 64).astype(BF))

    # --- edge partitioning ---
    core_of = rows // PPC
    dloc = rows - core_of * PPC
    t_of = dloc // 128
    rel_of = (dloc % 128).astype(np.float32)
    srcflat = flat_of(cols)
    r_of = srcflat // WINDOW
    lidx_of = (srcflat - r_of * WINDOW).astype(np.int16)

    # cell (t, r) edge lists per core
    cell_edges = [[[None] * NRANGE for _ in range(TPC)] for _ in range(NCORES)]
    for c in range(NCORES):
        m = core_of == c
        key = t_of[m] * NRANGE + r_of[m]
        order = np.argsort(key, kind="stable")
        eidx = np.nonzero(m)[0][order]
        k = key[order]
        bounds = np.searchsorted(k, np.arange(TPC * NRANGE + 1))
        for t in range(TPC):
            for r in range(NRANGE):
                a, b = bounds[t * NRANGE + r], bounds[t * NRANGE + r + 1]
                cell_edges[c][t][r] = eidx[a:b]

    # uniform chunk counts per (t, r): max over cores
    nchunk = np.zeros((TPC, NRANGE), np.int32)
    for t in range(TPC):
        for r in range(NRANGE):
            mx = max(len(cell_edges[c][t][r]) for c in range(NCORES))
            nchunk[t, r] = -(-mx // 128)

    # groups of tiles
    GT = 14
    groups = [list(range(a, min(a + GT, TPC))) for a in range(0, TPC, GT)]

    # consumption order: q index over (g, t-major, r, j); gather order per (g, r)
    # chunk (t, r, j) -> (q, gather slot within (g,r))
    NC = int(nchunk.sum())
    chunk_q = {}
    subk = {}       # (g, r) -> number of chunks in that gather
    slot_of = {}    # (t, r, j) -> slot in its (g, r) gather
    q = 0
    for gi, ts in enumerate(groups):
        for r in range(NRANGE):
            s = 0
            for t in ts:
                for j in range(nchunk[t, r]):
                    slot_of[(t, r, j)] = s
                    s += 1
            subk[(gi, r)] = s
        for r in range(NRANGE):
            for t in ts:
                for j in range(nchunk[t, r]):
                    chunk_q[(t, r, j)] = q
                    q += 1
    assert q == NC

    # per-core edge metadata arrays
    relv = np.zeros((NCORES, 128, NC), np.float32)
    valv = np.zeros((NCORES, 128, NC), np.float32)
    gidx_parts = {c: [] for c in range(NCORES)}   # per (g, r) int16 arrays
    gidx_off = {}                                  # (g, r) -> col offset in DRAM [128, ./16]
    off16 = 0
    for gi, ts in enumerate(groups):
        for r in range(NRANGE):
            sk = subk[(gi, r)]
            if sk == 0:
                continue
            gidx_off[(gi, r)] = off16
            off16 += sk * 8
            for c in range(NCORES):
                arr = np.zeros(sk * 128, np.int16)
                for t in ts:
                    for j in range(nchunk[t, r]):
                        s = slot_of[(t, r, j)]
                        e = cell_edges[c][t][r][j * 128:(j + 1) * 128]
                        ne = len(e)
                        arr[s * 128:s * 128 + ne] = lidx_of[e]
                        qq = chunk_q[(t, r, j)]
                        relv[c, :ne, qq] = rel_of[e]
                        valv[c, :ne, qq] = vals[e]
                gidx_parts[c].append(arr)
    gidx = [
        np.concatenate([_wrap16(a) for a in gidx_parts[c]], axis=1)
        for c in range(NCORES)
    ]
    TOT16 = gidx[0].shape[1]

    # --- weights with bias folded as extra row, bf16 ---
    wmats = {}
    for l in range(3):
        for nm in ("gc", "bi"):
            W = np.asarray(inputs[f"W_{nm}{l}"]).astype(np.float32)
            b = np.asarray(inputs[f"b_{nm}{l}"]).astype(np.float32)
            wmats[f"w_{nm}{l}"] = np.concatenate([W, b.reshape(1, -1)], 0).astype(BF)

    # --- scoring ---
    # staging position for batch b: partition b%128, col 3*(b//128)+{0,1,2}
    def spos(b, which):
        return (b % 128) + 256 * (3 * (b // 128) + which)

    sg_idx, sg_dst, sg_cnt = [], [], {}
    # bucket (by range r) sizes: max over cores, rounded to 128
    all_nodes = []
    for c in range(NCORES):
        u = users[c * BPC:(c + 1) * BPC].astype(np.int64)
        p = NU + pos_items[c * BPC:(c + 1) * BPC].astype(np.int64)
        n = NU + neg_items[c * BPC:(c + 1) * BPC].astype(np.int64)
        nodes = np.stack([u, p, n], 1).ravel()       # b-major, (u,p,n)
        which = np.tile(np.array([0, 1, 2]), BPC)
        bb = np.repeat(np.arange(BPC), 3)
        fl = flat_of(nodes)
        all_nodes.append((fl, spos(bb, which)))
    for r in range(NRANGE):
        mx = max(((fl // WINDOW) == r).sum() for fl, _ in all_nodes)
        sg_cnt[r] = int(-(-max(mx, 1) // 128) * 128)
    for c in range(NCORES):
        fl, sp = all_nodes[c]
        iparts, dparts = [], []
        padc = 0
        for r in range(NRANGE):
            m = (fl // WINDOW) == r
            cnt = int(m.sum())
            tot = sg_cnt[r]
            gi16 = np.zeros(tot, np.int16)
            gd16 = np.zeros(tot, np.int16)
            gi16[:cnt] = (fl[m] - r * WINDOW).astype(np.int16)
            gd16[:cnt] = sp[m].astype(np.int16)
            for i in range(cnt, tot):               # unique parity-1 dump slots
                gd16[i] = 128 + (padc % 128) + 256 * (padc // 128)
                padc += 1
            iparts.append(_wrap16(gi16))
            dparts.append(_wrap16(gd16))
        sg_idx.append(np.concatenate(iparts, 1))
        sg_dst.append(np.concatenate(dparts, 1))
    STOT16 = sg_idx[0].shape[1]

    iota = np.tile(np.arange(128, dtype=np.float32), (128, 1)).astype(BF)
    ident = np.eye(128, dtype=np.float32).astype(BF)

    PSTREAM = os.environ.get("KGAT_PSTREAM", "1") == "1"
    in_maps = []
    qi = np.broadcast_to(np.arange(NC)[None, :], (128, NC))
    pi = np.broadcast_to(np.arange(128)[:, None], (128, NC))
    t0dup = np.concatenate([ego0, ego0], axis=1).astype(BF)   # [NP, 128]
    for c in range(NCORES):
        m = dict(
            t0=t0dup,
            ego0sb=ego0sb[c],
            gidx=gidx[c],
            sgidx=sg_idx[c],
            sgdst=sg_dst[c],
            iota=iota,
            ident=ident,
        )
        if PSTREAM:
            pm = np.zeros((NC, 128, 128), np.float32)
            pm[qi.ravel(), pi.ravel(), relv[c].astype(np.int64).ravel()] = \
                valv[c].ravel()
            m["pmat"] = np.ascontiguousarray(
                pm.transpose(1, 0, 2).reshape(128, NC * 128)).astype(BF)
        else:
            m["relv"] = relv[c].astype(np.float32)
            m["valv"] = valv[c].astype(np.float32)
        m.update(wmats)
        in_maps.append(m)

    meta = dict(
        N=N, NP=NP, PPC=PPC, TPC=TPC, NRANGE=NRANGE, NC=NC,
        BPC=BPC, B3=B3, SC=SC, dins=dins, douts=douts, D0=D0,
        groups=groups, nchunk=nchunk, subk=subk, slot_of=slot_of,
        chunk_q=chunk_q, gidx_off=gidx_off, TOT16=TOT16,
        sg_cnt=sg_cnt, STOT16=STOT16, B=B,
    )
    return meta, in_maps


def build_program(meta):
    f32, bf16, i16 = mybir.dt.float32, mybir.dt.bfloat16, mybir.dt.int16
    NP, PPC, TPC = meta["NP"], meta["PPC"], meta["TPC"]
    NRANGE, NC = meta["NRANGE"], meta["NC"]
    B3, SC = meta["B3"], meta["SC"]
    dins, douts = meta["dins"], meta["douts"]
    groups, nchunk = meta["groups"], meta["nchunk"]
    subk, slot_of, chunk_q = meta["subk"], meta["slot_of"], meta["chunk_q"]
    gidx_off, sg_cnt = meta["gidx_off"], meta["sg_cnt"]

    SP = os.environ.get("KGAT_SP", "0") == "1"
    NQ = int(os.environ.get("KGAT_NQ", "4"))
    SCRATCH = int(os.environ.get("KGAT_SCRATCH", "32768"))
    PSTREAM = os.environ.get("KGAT_PSTREAM", "1") == "1"
    PBLK = int(os.environ.get("KGAT_PBLK", "32"))
    S0STEP = int(os.environ.get("KGAT_S0", "16"))
    nc = bacc.Bacc(num_swdge_queues=NQ, dynamic_dma_scratch_size=SCRATCH)

    t0 = nc.dram_tensor("t0", [NP, 128], bf16, kind="ExternalInput")
    ego0sb_in = nc.dram_tensor("ego0sb", [128, TPC * 64], bf16, kind="ExternalInput")
    if PSTREAM:
        pmat_in = nc.dram_tensor("pmat", [128, NC * 128], bf16, kind="ExternalInput")
    else:
        relv_in = nc.dram_tensor("relv", [128, NC], f32, kind="ExternalInput")
        valv_in = nc.dram_tensor("valv", [128, NC], f32, kind="ExternalInput")
    gidx_in = nc.dram_tensor("gidx", [128, meta["TOT16"]], i16, kind="ExternalInput")
    sgidx_in = nc.dram_tensor("sgidx", [128, meta["STOT16"]], i16, kind="ExternalInput")
    sgdst_in = nc.dram_tensor("sgdst", [128, meta["STOT16"]], i16, kind="ExternalInput")
    iota_in = nc.dram_tensor("iota", [128, 128], bf16, kind="ExternalInput")
    ident_in = nc.dram_tensor("ident", [128, 128], bf16, kind="ExternalInput")
    w_in = {}
    for l in range(3):
        for nm in ("gc", "bi"):
            w_in[f"{nm}{l}"] = nc.dram_tensor(
                f"w_{nm}{l}", [dins[l] + 1, douts[l]], bf16, kind="ExternalInput")
    scores_out = nc.dram_tensor("scores", [128, 2 * B3], f32, kind="ExternalOutput")

    with tile.TileContext(nc) as tc:
        with (
            tc.tile_pool(name="const", bufs=1) as cpool,
            tc.tile_pool(name="big", bufs=1) as bigp,
            tc.tile_pool(name="gf", bufs=4) as gfp,
            tc.tile_pool(name="gb", bufs=3) as gbp,
            tc.tile_pool(name="pp", bufs=6) as ppool,
            tc.tile_pool(name="pm", bufs=3) as pmp,
            tc.tile_pool(name="dense", bufs=2) as dpool,
            tc.tile_pool(name="psA", bufs=4, space="PSUM") as psA,
            tc.tile_pool(name="psB", bufs=2, space="PSUM") as psB,
            tc.tile_pool(name="psC", bufs=2, space="PSUM") as psC,
            tc.tile_pool(name="dram", bufs=1, space="DRAM") as dram,
        ):
            # ---- constants ----
            iota_t = cpool.tile([128, 128], bf16, tag="iota")
            nc.sync.dma_start(iota_t[:], iota_in[:])
            ident_t = cpool.tile([128, 128], bf16, tag="ident")
            nc.sync.dma_start(ident_t[:], ident_in[:])
            w_t = {}
            for l in range(3):
                for nm in ("gc", "bi"):
                    w = cpool.tile([dins[l] + 1, douts[l]], bf16, tag=f"w{nm}{l}")
                    nc.sync.dma_start(w[:], w_in[f"{nm}{l}"][:])
                    w_t[f"{nm}{l}"] = w
            if not PSTREAM:
                relv_t = cpool.tile([128, NC], f32, tag="relv")
                nc.sync.dma_start(relv_t[:], relv_in[:])
                valv_t = cpool.tile([128, NC], f32, tag="valv")
                nc.sync.dma_start(valv_t[:], valv_in[:])

            # ---- persistent big tiles ----
            ego_sb = bigp.tile([128, TPC * 64], bf16, tag="ego")
            nc.sync.dma_start(ego_sb[:], ego0sb_in[:])
            side_sb = bigp.tile([128, TPC * 64], bf16, tag="side")
            inv_sb = [bigp.tile([128, TPC], f32, tag=f"inv{l}", name=f"inv{l}")
                      for l in range(3)]

            # ---- DRAM tables / pieces (bf16, rows [feat|feat] = 256B) ----
            tables = [t0]
            pieces = []
            for l in range(3):
                T = dram.tile([NP, 128], bf16, tag=f"T{l+1}", name=f"T{l+1}", addr_space="Shared")
                tables.append(T)
                pieces.append(dram.tile([PPC, 128], bf16, tag=f"piece{l+1}", name=f"piece{l+1}"))

            # ================= layers =================
            gq = [0]
            NLAYERS = int(os.environ.get("KGAT_LAYERS", "3"))
            SKIP_AG = os.environ.get("KGAT_SKIP_AG", "0") == "1"
            SKIP_SCORE = os.environ.get("KGAT_SKIP_SCORE", "0") == "1"
            SKIP_EDGE = os.environ.get("KGAT_SKIP_EDGE", "0") == "1"
            SKIP_ONEHOT = os.environ.get("KGAT_SKIP_ONEHOT", "0") == "1"
            SKIP_MM = os.environ.get("KGAT_SKIP_MM", "0") == "1"
            NBLK = -(-NC // PBLK)

            def pm_cols(b):
                return min(PBLK * 128, NC * 128 - b * PBLK * 128)

            for l in range(NLAYERS):
                din, dout = dins[l], douts[l]
                Tsrc = tables[l]

                pm_blocks = {}
                if PSTREAM and not (SKIP_EDGE or SKIP_ONEHOT):
                    for b in range(min(2, NBLK)):
                        pmt = pmp.tile([128, PBLK * 128], bf16, tag="pmb")
                        nc.sync.dma_start(
                            pmt[:, :pm_cols(b)],
                            pmat_in[:, b * PBLK * 128:b * PBLK * 128 + pm_cols(b)])
                        pm_blocks[b] = pmt

                for gi, ts in enumerate(groups):
                    gbase = ts[0]
                    Tg = len(ts)
                    if SKIP_EDGE:
                        nc.vector.memset(side_sb[:, gbase * 64:(gbase + Tg) * 64], 0.0)
                    # ---- r-major: gather range r, matmul its chunks into
                    # per-tile PSUM accumulators; gather r+1 overlaps ----
                    ps_t = {}
                    done_t = {t: 0 for t in ts}
                    tot_t = {t: int(nchunk[t].sum()) for t in ts}
                    if not SKIP_EDGE:
                        nbank = -(-Tg // 8)
                        banks = [psA.tile([128, 8, 64], f32, tag="psb",
                                          name=f"psb{gi}_{bi}")
                                 for bi in range(nbank)]
                        for ti, t in enumerate(ts):
                            if tot_t[t] > 0 and not SKIP_ONEHOT:
                                ps_t[t] = banks[ti // 8][:, ti % 8, :]
                            else:
                                nc.vector.memset(
                                    side_sb[:, t * 64:t * 64 + din], 0.0)
                    for r in range(NRANGE):
                        if SKIP_EDGE:
                            break
                        sk = subk[(gi, r)]
                        if sk == 0:
                            continue
                        wsz = min(WINDOW, NP - r * WINDOW)
                        off = gidx_off[(gi, r)]
                        gb = gbp.tile([128, sk, 128], bf16, tag="gb")
                        for s0 in range(0, sk, S0STEP):
                            skc = min(S0STEP, sk - s0)
                            idxt = gfp.tile([128, skc * 8], i16, tag="idx")
                            nc.sync.dma_start(
                                idxt[:], gidx_in[:, off + s0 * 8:off + (s0 + skc) * 8])
                            nc.gpsimd.dma_gather(
                                out_ap=gb[:, s0:s0 + skc, :],
                                in_ap=Tsrc[r * WINDOW:r * WINDOW + wsz],
                                idxs_ap=idxt[:], num_idxs=skc * 128,
                                num_idxs_reg=skc * 128, elem_size=128,
                                single_packet=SP, queue_num=(gq[0] % NQ),
                            )
                            gq[0] += 1
                        if SKIP_ONEHOT:
                            continue
                        for t in ts:
                            for j in range(int(nchunk[t, r])):
                                qq = chunk_q[(t, r, j)]
                                s = slot_of[(t, r, j)]
                                if PSTREAM:
                                    b = qq // PBLK
                                    if b + 2 < NBLK and (b + 2) not in pm_blocks:
                                        pmt = pmp.tile(
                                            [128, PBLK * 128], bf16, tag="pmb")
                                        nc.sync.dma_start(
                                            pmt[:, :pm_cols(b + 2)],
                                            pmat_in[:, (b + 2) * PBLK * 128:
                                                    (b + 2) * PBLK * 128
                                                    + pm_cols(b + 2)])
                                        pm_blocks[b + 2] = pmt
                                    P_ap = pm_blocks[b][
                                        :, (qq - b * PBLK) * 128:
                                        (qq - b * PBLK) * 128 + 128]
                                else:
                                    P = ppool.tile([128, 128], bf16, tag="P")
                                    nc.vector.tensor_scalar(
                                        out=P[:], in0=iota_t[:],
                                        scalar1=relv_t[:, qq:qq + 1],
                                        scalar2=valv_t[:, qq:qq + 1],
                                        op0=ALU.is_equal, op1=ALU.mult,
                                    )
                                    P_ap = P[:]
                                if not SKIP_MM:
                                    nc.tensor.matmul(
                                        ps_t[t][:, :din], lhsT=P_ap,
                                        rhs=gb[:, s, :din],
                                        start=(done_t[t] == 0),
                                        stop=(done_t[t] == tot_t[t] - 1),
                                    )
                                done_t[t] += 1
                    for t in ts:
                        if SKIP_EDGE:
                            break
                        if t not in ps_t:
                            continue
                        if SKIP_MM:
                            nc.vector.memset(side_sb[:, t * 64:t * 64 + din], 0.0)
                        else:
                            nc.scalar.copy(out=side_sb[:, t * 64:t * 64 + din],
                                           in_=ps_t[t][:, :din])

                    # ---- dense phase for this group ----
                    sl3 = lambda big, w: big[:, gbase * 64:(gbase + Tg) * 64].rearrange(
                        "p (t d) -> p t d", d=64)[:, :, :w]
                    plus = dpool.tile([128, Tg, din + 1], bf16, tag="plus")
                    nc.vector.tensor_tensor(out=plus[:, :, :din], in0=sl3(side_sb, din),
                                            in1=sl3(ego_sb, din), op=ALU.add)
                    nc.vector.memset(plus[:, :, din], 1.0)
                    times = dpool.tile([128, Tg, din + 1], bf16, tag="times")
                    nc.vector.tensor_tensor(out=times[:, :, :din], in0=sl3(side_sb, din),
                                            in1=sl3(ego_sb, din), op=ALU.mult)
                    nc.vector.memset(times[:, :, din], 1.0)
                    bo = {}
                    for bname, src, wkey in (("p", plus, f"gc{l}"), ("b", times, f"bi{l}")):
                        out_b = dpool.tile([128, Tg * dout], f32, tag=f"bo{bname}")
                        for ti in range(Tg):
                            tp = psB.tile([din + 1, 128], bf16, tag="tp")
                            nc.tensor.transpose(tp[:], src[:, ti, :], ident_t[:])
                            xt = ppool.tile([din + 1, 128], bf16, tag="xt")
                            nc.vector.tensor_copy(out=xt[:], in_=tp[:])
                            mo = psC.tile([128, dout], f32, tag="mo")
                            nc.tensor.matmul(mo[:], lhsT=xt[:], rhs=w_t[wkey][:],
                                             start=True, stop=True)
                            ob = out_b[:, ti * dout:(ti + 1) * dout]
                            nc.scalar.activation(ob, mo[:], ACT.Copy, scale=0.01)
                            nc.vector.tensor_tensor(out=ob, in0=ob, in1=mo[:],
                                                    op=ALU.max)
                        bo[bname] = out_b
                    out_g = dpool.tile([128, Tg * 64], f32, tag="outg")
                    nc.vector.memset(out_g[:], 0.0)
                    og3 = out_g[:].rearrange("p (t d) -> p t d", d=64)[:, :, :dout]
                    nc.vector.tensor_tensor(out=og3, in0=bo["p"][:], in1=bo["b"][:],
                                            op=ALU.add)
                    # l2 norm factors
                    sq = dpool.tile([128, Tg * dout], f32, tag="sq")
                    nc.vector.tensor_tensor(out=sq[:], in0=og3, in1=og3, op=ALU.mult)
                    ssum = dpool.tile([128, Tg], f32, tag="ssum")
                    nc.vector.reduce_sum(
                        out=ssum[:], in_=sq[:].rearrange("p (t d) -> p t d", d=dout),
                        axis=mybir.AxisListType.X)
                    nrm = dpool.tile([128, Tg], f32, tag="nrm")
                    nc.scalar.activation(nrm[:], ssum[:], ACT.Sqrt)
                    nc.vector.tensor_scalar_max(out=nrm[:], in0=nrm[:], scalar1=1e-12)
                    nc.vector.reciprocal(inv_sb[l][:, gbase:gbase + Tg], nrm[:])
                    if l == 2:
                        og64 = out_g[:].rearrange("p (t d) -> p t d", d=64)
                        for i in range(3):
                            nc.vector.tensor_copy(
                                out=og64[:, :, 16 + i],
                                in_=inv_sb[i][:, gbase:gbase + Tg])
                    # update ego (bf16) and store duplicated bf16 piece
                    nc.vector.tensor_copy(
                        out=ego_sb[:, gbase * 64:(gbase + Tg) * 64], in_=out_g[:])
                    dup = dpool.tile([128, Tg, 128], bf16, tag="dup")
                    og64f = out_g[:].rearrange("p (t d) -> p t d", d=64)
                    nc.scalar.copy(out=dup[:, :, 0:64], in_=og64f)
                    nc.vector.tensor_copy(out=dup[:, :, 64:128], in_=og64f)
                    dst = pieces[l][:].rearrange("(p t) d -> p (t d)", p=128)
                    nc.sync.dma_start(
                        dst[:, gbase * 128:(gbase + Tg) * 128],
                        dup[:].rearrange("p t d -> p (t d)"))

                if not SKIP_AG:
                    nc.gpsimd.collective_compute(
                        "AllGather", ALU.bypass,
                        replica_groups=[list(range(NCORES))],
                        ins=[pieces[l].opt()], outs=[tables[l + 1].opt()],
                    )

            # ================= scoring =================
            if SKIP_SCORE:
                zz = dpool.tile([128, 2 * B3], f32, tag="zz")
                nc.vector.memset(zz[:], 0.0)
                nc.sync.dma_start(scores_out[:], zz[:])
            stage = []
            peer = None
            for ti in range(4) if not SKIP_SCORE else []:
                own = bigp.tile([128, SC, 128], bf16, tag=f"stown{ti}")
                if peer is None:
                    peer = bigp.tile([128, SC, 128], bf16, tag="stpeer")
                    nc.vector.memset(peer[:], 0.0)
                nc.vector.memset(own[:], 0.0)
                stage.append((own, peer))
            soff = 0
            for r in range(NRANGE) if not SKIP_SCORE else []:
                tot = sg_cnt[r]
                gi_t = gfp.tile([128, tot // 16], i16, tag="sgi")
                nc.sync.dma_start(gi_t[:], sgidx_in[:, soff:soff + tot // 16])
                gd_t = gfp.tile([128, tot // 16], i16, tag="sgd")
                nc.sync.dma_start(gd_t[:], sgdst_in[:, soff:soff + tot // 16])
                soff += tot // 16
                wsz = min(WINDOW, NP - r * WINDOW)
                for ti in range(4):
                    gf = gfp.tile([128, tot // 128, 128], bf16, tag="sgf")
                    nc.gpsimd.dma_gather(
                        out_ap=gf[:], in_ap=tables[ti][r * WINDOW:r * WINDOW + wsz],
                        idxs_ap=gi_t[:], num_idxs=tot, num_idxs_reg=tot,
                        elem_size=128, single_packet=SP,
                    )
                    own, peer = stage[ti]
                    nc.gpsimd.dma_scatter_add(
                        out_ap=own[:], in_ap=gf[:], idxs_ap=gd_t[:],
                        num_idxs=tot, num_idxs_reg=tot, elem_size=128,
                        sbuf_tokens_per_rank=128, parity_reg=0,
                        out_ap_other=peer[:], single_packet=False,
                    )
            # dots
            dls = [meta["D0"]] + douts
            acc = {}
            if SKIP_SCORE:
                dls = []
            for which, o1 in (("pos", 128), ("neg", 256)) if not SKIP_SCORE else []:
                total = dpool.tile([128, B3], f32, tag=f"tot{which}")
                for ti in range(4):
                    own = stage[ti][0]
                    dl = dls[ti]
                    u = own[:].rearrange("p c d -> p (c d)").rearrange(
                        "p (j x) -> p j x", x=384)[:, :, 0:dl]
                    v = own[:].rearrange("p c d -> p (c d)").rearrange(
                        "p (j x) -> p j x", x=384)[:, :, o1:o1 + dl]
                    prod = dpool.tile([128, B3 * dl], f32, tag="prod")
                    nc.vector.tensor_tensor(out=prod[:], in0=u, in1=v, op=ALU.mult)
                    d = dpool.tile([128, B3], f32, tag=f"dot{ti}{which}")
                    nc.vector.reduce_sum(
                        out=d[:], in_=prod[:].rearrange("p (j d) -> p j d", d=dl),
                        axis=mybir.AxisListType.X)
                    acc[(ti, which)] = d
                own3 = stage[3][0]
                for ti in range(1, 4):
                    col = 16 + ti - 1
                    iu = own3[:].rearrange("p c d -> p (c d)").rearrange(
                        "p (j x) -> p j x", x=384)[:, :, col]
                    iv = own3[:].rearrange("p c d -> p (c d)").rearrange(
                        "p (j x) -> p j x", x=384)[:, :, o1 + col]
                    d = acc[(ti, which)]
                    nc.vector.tensor_tensor(out=d[:], in0=d[:], in1=iu, op=ALU.mult)
                    nc.vector.tensor_tensor(out=d[:], in0=d[:], in1=iv, op=ALU.mult)
                nc.vector.tensor_tensor(out=total[:], in0=acc[(0, which)][:],
                                        in1=acc[(1, which)][:], op=ALU.add)
                nc.vector.tensor_tensor(out=total[:], in0=total[:],
                                        in1=acc[(2, which)][:], op=ALU.add)
                nc.vector.tensor_tensor(out=total[:], in0=total[:],
                                        in1=acc[(3, which)][:], op=ALU.add)
                acc[which] = total
            if not SKIP_SCORE:
                outt = dpool.tile([128, 2 * B3], f32, tag="outt")
                nc.vector.tensor_copy(out=outt[:, :B3], in_=acc["pos"][:])
                nc.vector.tensor_copy(out=outt[:, B3:], in_=acc["neg"][:])
                nc.sync.dma_start(scores_out[:], outt[:])

    nc.compile()
    return nc


def kernel(**inputs):
    meta, in_maps = build_host_data(inputs)
    nc = build_program(meta)
    trace = os.environ.get("KGAT_TRACE", "0") == "1"
    rr = run_bass_kernel_spmd(nc, in_maps, list(range(NCORES)), trace=trace)
    if trace and rr.exec_time_ns is not None:
        print(f"HW exec time: {rr.exec_time_ns} ns")
    if trace and rr.profile_json is not None:
        import json
        with open("/tmp/kgat_profile.json", "w") as f:
            json.dump(rr.profile_json, f)
    res = rr.results
    B3, BPC, B = meta["B3"], meta["BPC"], meta["B"]
    out = np.zeros((B, 2), np.float32)
    for c in range(NCORES):
        sc = res[c]["scores"]                       # [128, 2*B3]
        pos = sc[:, :B3]                            # [128, B3] (partition, jj)
        neg = sc[:, B3:]
        b = np.arange(BPC)
        out[c * BPC + b, 0] = pos[b % 128, b // 128]
        out[c * BPC + b, 1] = neg[b % 128, b // 128]
    return out



# revision 30
# speedup vs baseline: 1.0832x; 1.0832x over previous
"""KGAT forward kernel for 8 Trainium2 NeuronCores (Bass/Tile SPMD).

Strategy (dst-sharded graph parallel):
  - Nodes padded to NP = 8*PPC; core c owns rows [c*PPC, (c+1)*PPC).
  - Node tables T_l [NP, 64] f32 live replicated in each core's DRAM in an
    "image" layout: flat row index of node g = owner*PPC + (loc%128)*TPC + loc//128,
    so a core's piece maps 1:1 onto SBUF [128, TPC, 64] with node tile
    t = {128t+p} at column block t.
  - Edge phase per layer: per-edge source rows pulled with the custom
    dma_gather (int16 idx, windowed in 32768-row ranges), segment-sum by
    destination via one-hot matrices (iota is_equal) matmul-accumulated in
    PSUM per 128-destination tile.
  - Dense phase: X'=[X|1] per tile, PE transpose, bf16 matmul with
    W'=[W;b], leaky-relu on ACT, l2-norm factors kept (not applied) and
    packed into spare columns of T3 for use at scoring time.
  - AllGather (collective) replicates each new layer piece.
  - Scoring: batch shard per core; rows fetched with windowed gathers and
    re-ordered into batch order with unique-index dma_scatter_add into
    SBUF (parity-split); dot products on DVE.
"""
import sys
sys.path.insert(0, '/opt/trn_rl_repo')

import numpy as np
import ml_dtypes

import os
import concourse.bass as bass
import concourse.bacc as bacc
import concourse.tile as tile
from concourse import mybir
from concourse.bass_utils import run_bass_kernel_spmd

BF = ml_dtypes.bfloat16
NCORES = 8
WINDOW = 32768
ACT = mybir.ActivationFunctionType
ALU = mybir.AluOpType


def _wrap16(idx):
    """int16 idx array -> [128, n/16] wrapped+replicated layout."""
    n = len(idx)
    assert n % 16 == 0
    return np.tile(idx.reshape(n // 16, 16).T, (8, 1)).astype(np.int16)


def _img(loc, tpc):
    """local node id -> piece-flat image row."""
    return (loc % 128) * tpc + loc // 128


def build_host_data(inputs):
    """All host-side preprocessing. Returns (meta, in_maps)."""
    users = np.asarray(inputs["users"])
    pos_items = np.asarray(inputs["pos_items"])
    neg_items = np.asarray(inputs["neg_items"])
    rows = np.asarray(inputs["rows"]).astype(np.int64)
    cols = np.asarray(inputs["cols"]).astype(np.int64)
    vals = np.asarray(inputs["edge_vals"]).astype(np.float32)
    ue = np.asarray(inputs["user_embed"]).astype(np.float32)
    ee = np.asarray(inputs["entity_embed"]).astype(np.float32)

    NU, D0 = ue.shape
    NE = ee.shape[0]
    N = NU + NE
    B = users.shape[0]
    BPC = B // NCORES
    B3 = BPC // 128
    SC = 3 * B3

    PPC = -(-N // (NCORES * 128)) * 128
    NP = PPC * NCORES
    TPC = PPC // 128
    NRANGE = -(-NP // WINDOW)

    douts = [inputs[f"W_gc{l}"].shape[1] for l in range(3)]
    dins = [D0, douts[0], douts[1]]

    # --- node -> table flat row (image layout) ---
    def flat_of(g):
        c = g // PPC
        loc = g % PPC
        return c * PPC + _img(loc, TPC)

    # --- ego0 full table (image layout), fp32 ---
    ego0 = np.zeros((NP, 64), np.float32)
    allemb = np.concatenate([ue, ee], 0)
    ego0[flat_of(np.arange(N)), :D0] = allemb

    # --- per-core ego0 piece, SBUF image, bf16 ---
    ego0sb = []
    for c in range(NCORES):
        piece = ego0[c * PPC:(c + 1) * PPC]          # already image-ordered
        ego0sb.append(piece.reshape(128, TPC * 64).astype(BF))

    # --- edge partitioning ---
    core_of = rows // PPC
    dloc = rows - core_of * PPC
    t_of = dloc // 128
    rel_of = (dloc % 128).astype(np.float32)
    srcflat = flat_of(cols)
    r_of = srcflat // WINDOW
    lidx_of = (srcflat - r_of * WINDOW).astype(np.int16)

    # cell (t, r) edge lists per core
    cell_edges = [[[None] * NRANGE for _ in range(TPC)] for _ in range(NCORES)]
    for c in range(NCORES):
        m = core_of == c
        key = t_of[m] * NRANGE + r_of[m]
        order = np.argsort(key, kind="stable")
        eidx = np.nonzero(m)[0][order]
        k = key[order]
        bounds = np.searchsorted(k, np.arange(TPC * NRANGE + 1))
        for t in range(TPC):
            for r in range(NRANGE):
                a, b = bounds[t * NRANGE + r], bounds[t * NRANGE + r + 1]
                cell_edges[c][t][r] = eidx[a:b]

    # uniform chunk counts per (t, r): max over cores
    nchunk = np.zeros((TPC, NRANGE), np.int32)
    for t in range(TPC):
        for r in range(NRANGE):
            mx = max(len(cell_edges[c][t][r]) for c in range(NCORES))
            nchunk[t, r] = -(-mx // 128)

    # groups of tiles
    GT = 14
    groups = [list(range(a, min(a + GT, TPC))) for a in range(0, TPC, GT)]

    # consumption order: q index over (g, t-major, r, j); gather order per (g, r)
    # chunk (t, r, j) -> (q, gather slot within (g,r))
    NC = int(nchunk.sum())
    chunk_q = {}
    subk = {}       # (g, r) -> number of chunks in that gather
    slot_of = {}    # (t, r, j) -> slot in its (g, r) gather
    q = 0
    for gi, ts in enumerate(groups):
        for r in range(NRANGE):
            s = 0
            for t in ts:
                for j in range(nchunk[t, r]):
                    slot_of[(t, r, j)] = s
                    s += 1
            subk[(gi, r)] = s
        for r in range(NRANGE):
            for t in ts:
                for j in range(nchunk[t, r]):
                    chunk_q[(t, r, j)] = q
                    q += 1
    assert q == NC

    # per-core edge metadata arrays
    relv = np.zeros((NCORES, 128, NC), np.float32)
    valv = np.zeros((NCORES, 128, NC), np.float32)
    gidx_parts = {c: [] for c in range(NCORES)}   # per (g, r) int16 arrays
    gidx_off = {}                                  # (g, r) -> col offset in DRAM [128, ./16]
    off16 = 0
    for gi, ts in enumerate(groups):
        for r in range(NRANGE):
            sk = subk[(gi, r)]
            if sk == 0:
                continue
            gidx_off[(gi, r)] = off16
            off16 += sk * 8
            for c in range(NCORES):
                arr = np.zeros(sk * 128, np.int16)
                for t in ts:
                    for j in range(nchunk[t, r]):
                        s = slot_of[(t, r, j)]
                        e = cell_edges[c][t][r][j * 128:(j + 1) * 128]
                        ne = len(e)
                        arr[s * 128:s * 128 + ne] = lidx_of[e]
                        qq = chunk_q[(t, r, j)]
                        relv[c, :ne, qq] = rel_of[e]
                        valv[c, :ne, qq] = vals[e]
                gidx_parts[c].append(arr)
    gidx = [
        np.concatenate([_wrap16(a) for a in gidx_parts[c]], axis=1)
        for c in range(NCORES)
    ]
    TOT16 = gidx[0].shape[1]

    # --- weights with bias folded as extra row, bf16 ---
    wmats = {}
    for l in range(3):
        for nm in ("gc", "bi"):
            W = np.asarray(inputs[f"W_{nm}{l}"]).astype(np.float32)
            b = np.asarray(inputs[f"b_{nm}{l}"]).astype(np.float32)
            wmats[f"w_{nm}{l}"] = np.concatenate([W, b.reshape(1, -1)], 0).astype(BF)

    # --- scoring ---
    # staging position for batch b: partition b%128, col 3*(b//128)+{0,1,2}
    def spos(b, which):
        return (b % 128) + 256 * (3 * (b // 128) + which)

    sg_idx, sg_dst, sg_cnt = [], [], {}
    # bucket (by range r) sizes: max over cores, rounded to 128
    all_nodes = []
    for c in range(NCORES):
        u = users[c * BPC:(c + 1) * BPC].astype(np.int64)
        p = NU + pos_items[c * BPC:(c + 1) * BPC].astype(np.int64)
        n = NU + neg_items[c * BPC:(c + 1) * BPC].astype(np.int64)
        nodes = np.stack([u, p, n], 1).ravel()       # b-major, (u,p,n)
        which = np.tile(np.array([0, 1, 2]), BPC)
        bb = np.repeat(np.arange(BPC), 3)
        fl = flat_of(nodes)
        all_nodes.append((fl, spos(bb, which)))
    for r in range(NRANGE):
        mx = max(((fl // WINDOW) == r).sum() for fl, _ in all_nodes)
        sg_cnt[r] = int(-(-max(mx, 1) // 128) * 128)
    for c in range(NCORES):
        fl, sp = all_nodes[c]
        iparts, dparts = [], []
        padc = 0
        for r in range(NRANGE):
            m = (fl // WINDOW) == r
            cnt = int(m.sum())
            tot = sg_cnt[r]
            gi16 = np.zeros(tot, np.int16)
            gd16 = np.zeros(tot, np.int16)
            gi16[:cnt] = (fl[m] - r * WINDOW).astype(np.int16)
            gd16[:cnt] = sp[m].astype(np.int16)
            for i in range(cnt, tot):               # unique parity-1 dump slots
                gd16[i] = 128 + (padc % 128) + 256 * (padc // 128)
                padc += 1
            iparts.append(_wrap16(gi16))
            dparts.append(_wrap16(gd16))
        sg_idx.append(np.concatenate(iparts, 1))
        sg_dst.append(np.concatenate(dparts, 1))
    STOT16 = sg_idx[0].shape[1]

    iota = np.tile(np.arange(128, dtype=np.float32), (128, 1)).astype(BF)
    ident = np.eye(128, dtype=np.float32).astype(BF)

    PSTREAM = os.environ.get("KGAT_PSTREAM", "1") == "1"
    in_maps = []
    qi = np.broadcast_to(np.arange(NC)[None, :], (128, NC))
    pi = np.broadcast_to(np.arange(128)[:, None], (128, NC))
    t0dup = np.concatenate([ego0, ego0], axis=1).astype(BF)   # [NP, 128]
    for c in range(NCORES):
        m = dict(
            t0=t0dup,
            ego0sb=ego0sb[c],
            gidx=gidx[c],
            sgidx=sg_idx[c],
            sgdst=sg_dst[c],
            iota=iota,
            ident=ident,
        )
        if PSTREAM:
            pm = np.zeros((NC, 128, 128), np.float32)
            pm[qi.ravel(), pi.ravel(), relv[c].astype(np.int64).ravel()] = \
                valv[c].ravel()
            m["pmat"] = np.ascontiguousarray(
                pm.transpose(1, 0, 2).reshape(128, NC * 128)).astype(BF)
        else:
            m["relv"] = relv[c].astype(np.float32)
            m["valv"] = valv[c].astype(np.float32)
        m.update(wmats)
        in_maps.append(m)

    meta = dict(
        N=N, NP=NP, PPC=PPC, TPC=TPC, NRANGE=NRANGE, NC=NC,
        BPC=BPC, B3=B3, SC=SC, dins=dins, douts=douts, D0=D0,
        groups=groups, nchunk=nchunk, subk=subk, slot_of=slot_of,
        chunk_q=chunk_q, gidx_off=gidx_off, TOT16=TOT16,
        sg_cnt=sg_cnt, STOT16=STOT16, B=B,
    )
    return meta, in_maps


def _dma_gather_raw(gp, out_ap, in_ap, idxs_ap, num_idxs, elem_size, elem_step,
                    single_packet=False, queue_num=0):
    """Non-transpose HBM dma_gather allowing elem_size_bytes % 128 (the 256B
    restriction in bass.dma_gather only applies to transpose mode). Row
    stride (elem_step) must still be a 256B multiple."""
    from concourse import mybir as mb
    from concourse.ap_utils import ap_is_contiguous
    assert idxs_ap.dtype == mb.dt.int16
    assert in_ap.dtype == out_ap.dtype
    assert ap_is_contiguous(in_ap.ap[1:])
    assert ap_is_contiguous(out_ap.ap[1:])
    assert ap_is_contiguous(idxs_ap.ap[1:])
    assert in_ap.ap[0][0] == elem_step
    stride_bytes = elem_step * mb.dt.size(in_ap.dtype)
    assert stride_bytes % 256 == 0
    _in_ap = gp.lower_ap_dma(in_ap, for_custom_bir_dma=True)
    _idxs_ap = gp.lower_ap(idxs_ap)
    _out_ap = gp.lower_ap(out_ap)
    return gp.add_instruction(
        mb.InstDMAGatherAnt(
            name=gp.bass.get_next_instruction_name(),
            ins=[*_in_ap, _idxs_ap,
                 gp.lower_val_access(gp.to_reg(num_idxs))],
            outs=[_out_ap],
            transpose=False,
            num_idxs=num_idxs,
            elem_size=elem_size,
            stride_bytes_256=stride_bytes // 256,
            gen_mode=0,
            single_packet=single_packet,
            queue_num=queue_num,
            sbuf_tokens_per_rank=0,
            sbuf_free_dim_per_rank=0,
            sbuf_free_dim_pad_per_rank=0,
            sbuf_byte_offset=0,
        ))


def build_program(meta):
    f32, bf16, i16 = mybir.dt.float32, mybir.dt.bfloat16, mybir.dt.int16
    NP, PPC, TPC = meta["NP"], meta["PPC"], meta["TPC"]
    NRANGE, NC = meta["NRANGE"], meta["NC"]
    B3, SC = meta["B3"], meta["SC"]
    dins, douts = meta["dins"], meta["douts"]
    groups, nchunk = meta["groups"], meta["nchunk"]
    subk, slot_of, chunk_q = meta["subk"], meta["slot_of"], meta["chunk_q"]
    gidx_off, sg_cnt = meta["gidx_off"], meta["sg_cnt"]

    SP = os.environ.get("KGAT_SP", "0") == "1"
    NQ = int(os.environ.get("KGAT_NQ", "4"))
    SCRATCH = int(os.environ.get("KGAT_SCRATCH", "32768"))
    PSTREAM = os.environ.get("KGAT_PSTREAM", "1") == "1"
    PBLK = int(os.environ.get("KGAT_PBLK", "32"))
    S0STEP = int(os.environ.get("KGAT_S0", "16"))
    nc = bacc.Bacc(num_swdge_queues=NQ, dynamic_dma_scratch_size=SCRATCH)

    t0 = nc.dram_tensor("t0", [NP, 128], bf16, kind="ExternalInput")
    ego0sb_in = nc.dram_tensor("ego0sb", [128, TPC * 64], bf16, kind="ExternalInput")
    if PSTREAM:
        pmat_in = nc.dram_tensor("pmat", [128, NC * 128], bf16, kind="ExternalInput")
    else:
        relv_in = nc.dram_tensor("relv", [128, NC], f32, kind="ExternalInput")
        valv_in = nc.dram_tensor("valv", [128, NC], f32, kind="ExternalInput")
    gidx_in = nc.dram_tensor("gidx", [128, meta["TOT16"]], i16, kind="ExternalInput")
    sgidx_in = nc.dram_tensor("sgidx", [128, meta["STOT16"]], i16, kind="ExternalInput")
    sgdst_in = nc.dram_tensor("sgdst", [128, meta["STOT16"]], i16, kind="ExternalInput")
    iota_in = nc.dram_tensor("iota", [128, 128], bf16, kind="ExternalInput")
    ident_in = nc.dram_tensor("ident", [128, 128], bf16, kind="ExternalInput")
    w_in = {}
    for l in range(3):
        for nm in ("gc", "bi"):
            w_in[f"{nm}{l}"] = nc.dram_tensor(
                f"w_{nm}{l}", [dins[l] + 1, douts[l]], bf16, kind="ExternalInput")
    scores_out = nc.dram_tensor("scores", [128, 2 * B3], f32, kind="ExternalOutput")

    with tile.TileContext(nc) as tc:
        with (
            tc.tile_pool(name="const", bufs=1) as cpool,
            tc.tile_pool(name="big", bufs=1) as bigp,
            tc.tile_pool(name="gf", bufs=4) as gfp,
            tc.tile_pool(name="gb", bufs=3) as gbp,
            tc.tile_pool(name="pp", bufs=6) as ppool,
            tc.tile_pool(name="pm", bufs=3) as pmp,
            tc.tile_pool(name="dense", bufs=2) as dpool,
            tc.tile_pool(name="psA", bufs=4, space="PSUM") as psA,
            tc.tile_pool(name="psB", bufs=2, space="PSUM") as psB,
            tc.tile_pool(name="psC", bufs=2, space="PSUM") as psC,
            tc.tile_pool(name="dram", bufs=1, space="DRAM") as dram,
        ):
            # ---- constants ----
            iota_t = cpool.tile([128, 128], bf16, tag="iota")
            nc.sync.dma_start(iota_t[:], iota_in[:])
            ident_t = cpool.tile([128, 128], bf16, tag="ident")
            nc.sync.dma_start(ident_t[:], ident_in[:])
            w_t = {}
            for l in range(3):
                for nm in ("gc", "bi"):
                    w = cpool.tile([dins[l] + 1, douts[l]], bf16, tag=f"w{nm}{l}")
                    nc.sync.dma_start(w[:], w_in[f"{nm}{l}"][:])
                    w_t[f"{nm}{l}"] = w
            if not PSTREAM:
                relv_t = cpool.tile([128, NC], f32, tag="relv")
                nc.sync.dma_start(relv_t[:], relv_in[:])
                valv_t = cpool.tile([128, NC], f32, tag="valv")
                nc.sync.dma_start(valv_t[:], valv_in[:])

            # ---- persistent big tiles ----
            ego_sb = bigp.tile([128, TPC * 64], bf16, tag="ego")
            nc.sync.dma_start(ego_sb[:], ego0sb_in[:])
            side_sb = bigp.tile([128, TPC * 64], bf16, tag="side")
            inv_sb = [bigp.tile([128, TPC], f32, tag=f"inv{l}", name=f"inv{l}")
                      for l in range(3)]

            # ---- DRAM tables / pieces (bf16, rows [feat|feat] = 256B) ----
            tables = [t0]
            pieces = []
            for l in range(3):
                T = dram.tile([NP, 128], bf16, tag=f"T{l+1}", name=f"T{l+1}", addr_space="Shared")
                tables.append(T)
                pieces.append(dram.tile([PPC, 128], bf16, tag=f"piece{l+1}", name=f"piece{l+1}"))

            # ================= layers =================
            gq = [0]
            NLAYERS = int(os.environ.get("KGAT_LAYERS", "3"))
            SKIP_AG = os.environ.get("KGAT_SKIP_AG", "0") == "1"
            SKIP_SCORE = os.environ.get("KGAT_SKIP_SCORE", "0") == "1"
            SKIP_EDGE = os.environ.get("KGAT_SKIP_EDGE", "0") == "1"
            SKIP_ONEHOT = os.environ.get("KGAT_SKIP_ONEHOT", "0") == "1"
            SKIP_MM = os.environ.get("KGAT_SKIP_MM", "0") == "1"
            NBLK = -(-NC // PBLK)

            def pm_cols(b):
                return min(PBLK * 128, NC * 128 - b * PBLK * 128)

            for l in range(NLAYERS):
                din, dout = dins[l], douts[l]
                Tsrc = tables[l]

                pm_blocks = {}
                if PSTREAM and not (SKIP_EDGE or SKIP_ONEHOT):
                    for b in range(min(2, NBLK)):
                        pmt = pmp.tile([128, PBLK * 128], bf16, tag="pmb")
                        nc.sync.dma_start(
                            pmt[:, :pm_cols(b)],
                            pmat_in[:, b * PBLK * 128:b * PBLK * 128 + pm_cols(b)])
                        pm_blocks[b] = pmt

                for gi, ts in enumerate(groups):
                    gbase = ts[0]
                    Tg = len(ts)
                    if SKIP_EDGE:
                        nc.vector.memset(side_sb[:, gbase * 64:(gbase + Tg) * 64], 0.0)
                    # ---- r-major: gather range r, matmul its chunks into
                    # per-tile PSUM accumulators; gather r+1 overlaps ----
                    ps_t = {}
                    done_t = {t: 0 for t in ts}
                    tot_t = {t: int(nchunk[t].sum()) for t in ts}
                    if not SKIP_EDGE:
                        nbank = -(-Tg // 8)
                        banks = [psA.tile([128, 8, 64], f32, tag="psb",
                                          name=f"psb{gi}_{bi}")
                                 for bi in range(nbank)]
                        for ti, t in enumerate(ts):
                            if tot_t[t] > 0 and not SKIP_ONEHOT:
                                ps_t[t] = banks[ti // 8][:, ti % 8, :]
                            else:
                                nc.vector.memset(
                                    side_sb[:, t * 64:t * 64 + din], 0.0)
                    for r in range(NRANGE):
                        if SKIP_EDGE:
                            break
                        sk = subk[(gi, r)]
                        if sk == 0:
                            continue
                        wsz = min(WINDOW, NP - r * WINDOW)
                        off = gidx_off[(gi, r)]
                        gb = gbp.tile([128, sk, 64], bf16, tag="gb")
                        for s0 in range(0, sk, S0STEP):
                            skc = min(S0STEP, sk - s0)
                            idxt = gfp.tile([128, skc * 8], i16, tag="idx")
                            nc.sync.dma_start(
                                idxt[:], gidx_in[:, off + s0 * 8:off + (s0 + skc) * 8])
                            _dma_gather_raw(
                                nc.gpsimd,
                                out_ap=gb[:, s0:s0 + skc, :],
                                in_ap=Tsrc[r * WINDOW:r * WINDOW + wsz],
                                idxs_ap=idxt[:], num_idxs=skc * 128,
                                elem_size=64, elem_step=128,
                                single_packet=SP, queue_num=(gq[0] % NQ),
                            )
                            gq[0] += 1
                        if SKIP_ONEHOT:
                            continue
                        for t in ts:
                            for j in range(int(nchunk[t, r])):
                                qq = chunk_q[(t, r, j)]
                                s = slot_of[(t, r, j)]
                                if PSTREAM:
                                    b = qq // PBLK
                                    if b + 2 < NBLK and (b + 2) not in pm_blocks:
                                        pmt = pmp.tile(
                                            [128, PBLK * 128], bf16, tag="pmb")
                                        nc.sync.dma_start(
                                            pmt[:, :pm_cols(b + 2)],
                                            pmat_in[:, (b + 2) * PBLK * 128:
                                                    (b + 2) * PBLK * 128
                                                    + pm_cols(b + 2)])
                                        pm_blocks[b + 2] = pmt
                                    P_ap = pm_blocks[b][
                                        :, (qq - b * PBLK) * 128:
                                        (qq - b * PBLK) * 128 + 128]
                                else:
                                    P = ppool.tile([128, 128], bf16, tag="P")
                                    nc.vector.tensor_scalar(
                                        out=P[:], in0=iota_t[:],
                                        scalar1=relv_t[:, qq:qq + 1],
                                        scalar2=valv_t[:, qq:qq + 1],
                                        op0=ALU.is_equal, op1=ALU.mult,
                                    )
                                    P_ap = P[:]
                                if not SKIP_MM:
                                    nc.tensor.matmul(
                                        ps_t[t][:, :din], lhsT=P_ap,
                                        rhs=gb[:, s, :din],
                                        start=(done_t[t] == 0),
                                        stop=(done_t[t] == tot_t[t] - 1),
                                    )
                                done_t[t] += 1
                    for t in ts:
                        if SKIP_EDGE:
                            break
                        if t not in ps_t:
                            continue
                        if SKIP_MM:
                            nc.vector.memset(side_sb[:, t * 64:t * 64 + din], 0.0)
                        else:
                            nc.scalar.copy(out=side_sb[:, t * 64:t * 64 + din],
                                           in_=ps_t[t][:, :din])

                    # ---- dense phase for this group ----
                    sl3 = lambda big, w: big[:, gbase * 64:(gbase + Tg) * 64].rearrange(
                        "p (t d) -> p t d", d=64)[:, :, :w]
                    plus = dpool.tile([128, Tg, din + 1], bf16, tag="plus")
                    nc.vector.tensor_tensor(out=plus[:, :, :din], in0=sl3(side_sb, din),
                                            in1=sl3(ego_sb, din), op=ALU.add)
                    nc.vector.memset(plus[:, :, din], 1.0)
                    times = dpool.tile([128, Tg, din + 1], bf16, tag="times")
                    nc.vector.tensor_tensor(out=times[:, :, :din], in0=sl3(side_sb, din),
                                            in1=sl3(ego_sb, din), op=ALU.mult)
                    nc.vector.memset(times[:, :, din], 1.0)
                    bo = {}
                    for bname, src, wkey in (("p", plus, f"gc{l}"), ("b", times, f"bi{l}")):
                        out_b = dpool.tile([128, Tg * dout], f32, tag=f"bo{bname}")
                        for ti in range(Tg):
                            tp = psB.tile([din + 1, 128], bf16, tag="tp")
                            nc.tensor.transpose(tp[:], src[:, ti, :], ident_t[:])
                            xt = ppool.tile([din + 1, 128], bf16, tag="xt")
                            nc.vector.tensor_copy(out=xt[:], in_=tp[:])
                            mo = psC.tile([128, dout], f32, tag="mo")
                            nc.tensor.matmul(mo[:], lhsT=xt[:], rhs=w_t[wkey][:],
                                             start=True, stop=True)
                            ob = out_b[:, ti * dout:(ti + 1) * dout]
                            nc.scalar.activation(ob, mo[:], ACT.Copy, scale=0.01)
                            nc.vector.tensor_tensor(out=ob, in0=ob, in1=mo[:],
                                                    op=ALU.max)
                        bo[bname] = out_b
                    out_g = dpool.tile([128, Tg * 64], f32, tag="outg")
                    nc.vector.memset(out_g[:], 0.0)
                    og3 = out_g[:].rearrange("p (t d) -> p t d", d=64)[:, :, :dout]
                    nc.vector.tensor_tensor(out=og3, in0=bo["p"][:], in1=bo["b"][:],
                                            op=ALU.add)
                    # l2 norm factors
                    sq = dpool.tile([128, Tg * dout], f32, tag="sq")
                    nc.vector.tensor_tensor(out=sq[:], in0=og3, in1=og3, op=ALU.mult)
                    ssum = dpool.tile([128, Tg], f32, tag="ssum")
                    nc.vector.reduce_sum(
                        out=ssum[:], in_=sq[:].rearrange("p (t d) -> p t d", d=dout),
                        axis=mybir.AxisListType.X)
                    nrm = dpool.tile([128, Tg], f32, tag="nrm")
                    nc.scalar.activation(nrm[:], ssum[:], ACT.Sqrt)
                    nc.vector.tensor_scalar_max(out=nrm[:], in0=nrm[:], scalar1=1e-12)
                    nc.vector.reciprocal(inv_sb[l][:, gbase:gbase + Tg], nrm[:])
                    if l == 2:
                        og64 = out_g[:].rearrange("p (t d) -> p t d", d=64)
                        for i in range(3):
                            nc.vector.tensor_copy(
                                out=og64[:, :, 16 + i],
                                in_=inv_sb[i][:, gbase:gbase + Tg])
                    # update ego (bf16) and store duplicated bf16 piece
                    nc.vector.tensor_copy(
                        out=ego_sb[:, gbase * 64:(gbase + Tg) * 64], in_=out_g[:])
                    dup = dpool.tile([128, Tg, 128], bf16, tag="dup")
                    og64f = out_g[:].rearrange("p (t d) -> p t d", d=64)
                    nc.scalar.copy(out=dup[:, :, 0:64], in_=og64f)
                    nc.vector.tensor_copy(out=dup[:, :, 64:128], in_=og64f)
                    dst = pieces[l][:].rearrange("(p t) d -> p (t d)", p=128)
                    nc.sync.dma_start(
                        dst[:, gbase * 128:(gbase + Tg) * 128],
                        dup[:].rearrange("p t d -> p (t d)"))

                if not SKIP_AG:
                    nc.gpsimd.collective_compute(
                        "AllGather", ALU.bypass,
                        replica_groups=[list(range(NCORES))],
                        ins=[pieces[l].opt()], outs=[tables[l + 1].opt()],
                    )

            # ================= scoring =================
            if SKIP_SCORE:
                zz = dpool.tile([128, 2 * B3], f32, tag="zz")
                nc.vector.memset(zz[:], 0.0)
                nc.sync.dma_start(scores_out[:], zz[:])
            stage = []
            peer = None
            for ti in range(4) if not SKIP_SCORE else []:
                own = bigp.tile([128, SC, 128], bf16, tag=f"stown{ti}")
                if peer is None:
                    peer = bigp.tile([128, SC, 128], bf16, tag="stpeer")
                    nc.vector.memset(peer[:], 0.0)
                nc.vector.memset(own[:], 0.0)
                stage.append((own, peer))
            soff = 0
            for r in range(NRANGE) if not SKIP_SCORE else []:
                tot = sg_cnt[r]
                gi_t = gfp.tile([128, tot // 16], i16, tag="sgi")
                nc.sync.dma_start(gi_t[:], sgidx_in[:, soff:soff + tot // 16])
                gd_t = gfp.tile([128, tot // 16], i16, tag="sgd")
                nc.sync.dma_start(gd_t[:], sgdst_in[:, soff:soff + tot // 16])
                soff += tot // 16
                wsz = min(WINDOW, NP - r * WINDOW)
                for ti in range(4):
                    gf = gfp.tile([128, tot // 128, 128], bf16, tag="sgf")
                    nc.gpsimd.dma_gather(
                        out_ap=gf[:], in_ap=tables[ti][r * WINDOW:r * WINDOW + wsz],
                        idxs_ap=gi_t[:], num_idxs=tot, num_idxs_reg=tot,
                        elem_size=128, single_packet=SP,
                    )
                    own, peer = stage[ti]
                    nc.gpsimd.dma_scatter_add(
                        out_ap=own[:], in_ap=gf[:], idxs_ap=gd_t[:],
                        num_idxs=tot, num_idxs_reg=tot, elem_size=128,
                        sbuf_tokens_per_rank=128, parity_reg=0,
                        out_ap_other=peer[:], single_packet=False,
                    )
            # dots
            dls = [meta["D0"]] + douts
            acc = {}
            if SKIP_SCORE:
                dls = []
            for which, o1 in (("pos", 128), ("neg", 256)) if not SKIP_SCORE else []:
                total = dpool.tile([128, B3], f32, tag=f"tot{which}")
                for ti in range(4):
                    own = stage[ti][0]
                    dl = dls[ti]
                    u = own[:].rearrange("p c d -> p (c d)").rearrange(
                        "p (j x) -> p j x", x=384)[:, :, 0:dl]
                    v = own[:].rearrange("p c d -> p (c d)").rearrange(
                        "p (j x) -> p j x", x=384)[:, :, o1:o1 + dl]
                    prod = dpool.tile([128, B3 * dl], f32, tag="prod")
                    nc.vector.tensor_tensor(out=prod[:], in0=u, in1=v, op=ALU.mult)
                    d = dpool.tile([128, B3], f32, tag=f"dot{ti}{which}")
                    nc.vector.reduce_sum(
                        out=d[:], in_=prod[:].rearrange("p (j d) -> p j d", d=dl),
                        axis=mybir.AxisListType.X)
                    acc[(ti, which)] = d
                own3 = stage[3][0]
                for ti in range(1, 4):
                    col = 16 + ti - 1
                    iu = own3[:].rearrange("p c d -> p (c d)").rearrange(
                        "p (j x) -> p j x", x=384)[:, :, col]
                    iv = own3[:].rearrange("p c d -> p (c d)").rearrange(
                        "p (j x) -> p j x", x=384)[:, :, o1 + col]
                    d = acc[(ti, which)]
                    nc.vector.tensor_tensor(out=d[:], in0=d[:], in1=iu, op=ALU.mult)
                    nc.vector.tensor_tensor(out=d[:], in0=d[:], in1=iv, op=ALU.mult)
                nc.vector.tensor_tensor(out=total[:], in0=acc[(0, which)][:],
                                        in1=acc[(1, which)][:], op=ALU.add)
                nc.vector.tensor_tensor(out=total[:], in0=total[:],
                                        in1=acc[(2, which)][:], op=ALU.add)
                nc.vector.tensor_tensor(out=total[:], in0=total[:],
                                        in1=acc[(3, which)][:], op=ALU.add)
                acc[which] = total
            if not SKIP_SCORE:
                outt = dpool.tile([128, 2 * B3], f32, tag="outt")
                nc.vector.tensor_copy(out=outt[:, :B3], in_=acc["pos"][:])
                nc.vector.tensor_copy(out=outt[:, B3:], in_=acc["neg"][:])
                nc.sync.dma_start(scores_out[:], outt[:])

    nc.compile()
    return nc


def kernel(**inputs):
    meta, in_maps = build_host_data(inputs)
    nc = build_program(meta)
    trace = os.environ.get("KGAT_TRACE", "0") == "1"
    rr = run_bass_kernel_spmd(nc, in_maps, list(range(NCORES)), trace=trace)
    if trace and rr.exec_time_ns is not None:
        print(f"HW exec time: {rr.exec_time_ns} ns")
    if trace and rr.profile_json is not None:
        import json
        with open("/tmp/kgat_profile.json", "w") as f:
            json.dump(rr.profile_json, f)
    res = rr.results
    B3, BPC, B = meta["B3"], meta["BPC"], meta["B"]
    out = np.zeros((B, 2), np.float32)
    for c in range(NCORES):
        sc = res[c]["scores"]                       # [128, 2*B3]
        pos = sc[:, :B3]                            # [128, B3] (partition, jj)
        neg = sc[:, B3:]
        b = np.arange(BPC)
        out[c * BPC + b, 0] = pos[b % 128, b // 128]
        out[c * BPC + b, 1] = neg[b % 128, b // 128]
    return out



# revision 31
# speedup vs baseline: 1.1024x; 1.0177x over previous
"""KGAT forward kernel for 8 Trainium2 NeuronCores (Bass/Tile SPMD).

Strategy (dst-sharded graph parallel):
  - Nodes padded to NP = 8*PPC; core c owns rows [c*PPC, (c+1)*PPC).
  - Node tables T_l [NP, 64] f32 live replicated in each core's DRAM in an
    "image" layout: flat row index of node g = owner*PPC + (loc%128)*TPC + loc//128,
    so a core's piece maps 1:1 onto SBUF [128, TPC, 64] with node tile
    t = {128t+p} at column block t.
  - Edge phase per layer: per-edge source rows pulled with the custom
    dma_gather (int16 idx, windowed in 32768-row ranges), segment-sum by
    destination via one-hot matrices (iota is_equal) matmul-accumulated in
    PSUM per 128-destination tile.
  - Dense phase: X'=[X|1] per tile, PE transpose, bf16 matmul with
    W'=[W;b], leaky-relu on ACT, l2-norm factors kept (not applied) and
    packed into spare columns of T3 for use at scoring time.
  - AllGather (collective) replicates each new layer piece.
  - Scoring: batch shard per core; rows fetched with windowed gathers and
    re-ordered into batch order with unique-index dma_scatter_add into
    SBUF (parity-split); dot products on DVE.
"""
import sys
sys.path.insert(0, '/opt/trn_rl_repo')

import numpy as np
import ml_dtypes

import os
import concourse.bass as bass
import concourse.bacc as bacc
import concourse.tile as tile
from concourse import mybir
from concourse.bass_utils import run_bass_kernel_spmd

BF = ml_dtypes.bfloat16
NCORES = 8
WINDOW = 32768
ACT = mybir.ActivationFunctionType
ALU = mybir.AluOpType


def _wrap16(idx):
    """int16 idx array -> [128, n/16] wrapped+replicated layout."""
    n = len(idx)
    assert n % 16 == 0
    return np.tile(idx.reshape(n // 16, 16).T, (8, 1)).astype(np.int16)


def _img(loc, tpc):
    """local node id -> piece-flat image row."""
    return (loc % 128) * tpc + loc // 128


def build_host_data(inputs):
    """All host-side preprocessing. Returns (meta, in_maps)."""
    users = np.asarray(inputs["users"])
    pos_items = np.asarray(inputs["pos_items"])
    neg_items = np.asarray(inputs["neg_items"])
    rows = np.asarray(inputs["rows"]).astype(np.int64)
    cols = np.asarray(inputs["cols"]).astype(np.int64)
    vals = np.asarray(inputs["edge_vals"]).astype(np.float32)
    ue = np.asarray(inputs["user_embed"]).astype(np.float32)
    ee = np.asarray(inputs["entity_embed"]).astype(np.float32)

    NU, D0 = ue.shape
    NE = ee.shape[0]
    N = NU + NE
    B = users.shape[0]
    BPC = B // NCORES
    B3 = BPC // 128
    SC = 3 * B3

    PPC = -(-N // (NCORES * 128)) * 128
    NP = PPC * NCORES
    TPC = PPC // 128
    NRANGE = -(-NP // WINDOW)

    douts = [inputs[f"W_gc{l}"].shape[1] for l in range(3)]
    dins = [D0, douts[0], douts[1]]

    # --- node -> table flat row (image layout) ---
    def flat_of(g):
        c = g // PPC
        loc = g % PPC
        return c * PPC + _img(loc, TPC)

    # --- ego0 full table (image layout), fp32 ---
    ego0 = np.zeros((NP, 64), np.float32)
    allemb = np.concatenate([ue, ee], 0)
    ego0[flat_of(np.arange(N)), :D0] = allemb

    # --- per-core ego0 piece, SBUF image, bf16 ---
    ego0sb = []
    for c in range(NCORES):
        piece = ego0[c * PPC:(c + 1) * PPC]          # already image-ordered
        ego0sb.append(piece.reshape(128, TPC * 64).astype(BF))

    # --- edge partitioning ---
    core_of = rows // PPC
    dloc = rows - core_of * PPC
    t_of = dloc // 128
    rel_of = (dloc % 128).astype(np.float32)
    srcflat = flat_of(cols)
    r_of = srcflat // WINDOW
    lidx_of = (srcflat - r_of * WINDOW).astype(np.int16)

    # cell (t, r) edge lists per core
    cell_edges = [[[None] * NRANGE for _ in range(TPC)] for _ in range(NCORES)]
    for c in range(NCORES):
        m = core_of == c
        key = t_of[m] * NRANGE + r_of[m]
        order = np.argsort(key, kind="stable")
        eidx = np.nonzero(m)[0][order]
        k = key[order]
        bounds = np.searchsorted(k, np.arange(TPC * NRANGE + 1))
        for t in range(TPC):
            for r in range(NRANGE):
                a, b = bounds[t * NRANGE + r], bounds[t * NRANGE + r + 1]
                cell_edges[c][t][r] = eidx[a:b]

    # uniform chunk counts per (t, r): max over cores
    nchunk = np.zeros((TPC, NRANGE), np.int32)
    for t in range(TPC):
        for r in range(NRANGE):
            mx = max(len(cell_edges[c][t][r]) for c in range(NCORES))
            nchunk[t, r] = -(-mx // 128)

    # groups of tiles
    GT = 14
    groups = [list(range(a, min(a + GT, TPC))) for a in range(0, TPC, GT)]

    # consumption order: q index over (g, t-major, r, j); gather order per (g, r)
    # chunk (t, r, j) -> (q, gather slot within (g,r))
    NC = int(nchunk.sum())
    chunk_q = {}
    subk = {}       # (g, r) -> number of chunks in that gather
    slot_of = {}    # (t, r, j) -> slot in its (g, r) gather
    q = 0
    for gi, ts in enumerate(groups):
        for r in range(NRANGE):
            s = 0
            for t in ts:
                for j in range(nchunk[t, r]):
                    slot_of[(t, r, j)] = s
                    s += 1
            subk[(gi, r)] = s
        for r in range(NRANGE):
            for t in ts:
                for j in range(nchunk[t, r]):
                    chunk_q[(t, r, j)] = q
                    q += 1
    assert q == NC

    # per-core edge metadata arrays
    relv = np.zeros((NCORES, 128, NC), np.float32)
    valv = np.zeros((NCORES, 128, NC), np.float32)
    gidx_parts = {c: [] for c in range(NCORES)}   # per (g, r) int16 arrays
    gidx_off = {}                                  # (g, r) -> col offset in DRAM [128, ./16]
    off16 = 0
    for gi, ts in enumerate(groups):
        for r in range(NRANGE):
            sk = subk[(gi, r)]
            if sk == 0:
                continue
            gidx_off[(gi, r)] = off16
            off16 += sk * 8
            for c in range(NCORES):
                arr = np.zeros(sk * 128, np.int16)
                for t in ts:
                    for j in range(nchunk[t, r]):
                        s = slot_of[(t, r, j)]
                        e = cell_edges[c][t][r][j * 128:(j + 1) * 128]
                        ne = len(e)
                        arr[s * 128:s * 128 + ne] = lidx_of[e]
                        qq = chunk_q[(t, r, j)]
                        relv[c, :ne, qq] = rel_of[e]
                        valv[c, :ne, qq] = vals[e]
                gidx_parts[c].append(arr)
    gidx = [
        np.concatenate([_wrap16(a) for a in gidx_parts[c]], axis=1)
        for c in range(NCORES)
    ]
    TOT16 = gidx[0].shape[1]

    # --- weights with bias folded as extra row, bf16 ---
    wmats = {}
    for l in range(3):
        for nm in ("gc", "bi"):
            W = np.asarray(inputs[f"W_{nm}{l}"]).astype(np.float32)
            b = np.asarray(inputs[f"b_{nm}{l}"]).astype(np.float32)
            wmats[f"w_{nm}{l}"] = np.concatenate([W, b.reshape(1, -1)], 0).astype(BF)

    # --- scoring ---
    # staging position for batch b: partition b%128, col 3*(b//128)+{0,1,2}
    def spos(b, which):
        return (b % 128) + 256 * (3 * (b // 128) + which)

    sg_idx, sg_dst, sg_cnt = [], [], {}
    # bucket (by range r) sizes: max over cores, rounded to 128
    all_nodes = []
    for c in range(NCORES):
        u = users[c * BPC:(c + 1) * BPC].astype(np.int64)
        p = NU + pos_items[c * BPC:(c + 1) * BPC].astype(np.int64)
        n = NU + neg_items[c * BPC:(c + 1) * BPC].astype(np.int64)
        nodes = np.stack([u, p, n], 1).ravel()       # b-major, (u,p,n)
        which = np.tile(np.array([0, 1, 2]), BPC)
        bb = np.repeat(np.arange(BPC), 3)
        fl = flat_of(nodes)
        all_nodes.append((fl, spos(bb, which)))
    for r in range(NRANGE):
        mx = max(((fl // WINDOW) == r).sum() for fl, _ in all_nodes)
        sg_cnt[r] = int(-(-max(mx, 1) // 128) * 128)
    for c in range(NCORES):
        fl, sp = all_nodes[c]
        iparts, dparts = [], []
        padc = 0
        for r in range(NRANGE):
            m = (fl // WINDOW) == r
            cnt = int(m.sum())
            tot = sg_cnt[r]
            gi16 = np.zeros(tot, np.int16)
            gd16 = np.zeros(tot, np.int16)
            gi16[:cnt] = (fl[m] - r * WINDOW).astype(np.int16)
            gd16[:cnt] = sp[m].astype(np.int16)
            for i in range(cnt, tot):               # unique parity-1 dump slots
                gd16[i] = 128 + (padc % 128) + 256 * (padc // 128)
                padc += 1
            iparts.append(_wrap16(gi16))
            dparts.append(_wrap16(gd16))
        sg_idx.append(np.concatenate(iparts, 1))
        sg_dst.append(np.concatenate(dparts, 1))
    STOT16 = sg_idx[0].shape[1]

    iota = np.tile(np.arange(128, dtype=np.float32), (128, 1)).astype(BF)
    ident = np.eye(128, dtype=np.float32).astype(BF)

    PSTREAM = os.environ.get("KGAT_PSTREAM", "1") == "1"
    in_maps = []
    qi = np.broadcast_to(np.arange(NC)[None, :], (128, NC))
    pi = np.broadcast_to(np.arange(128)[:, None], (128, NC))
    t0dup = np.concatenate([ego0, ego0], axis=1).astype(BF)   # [NP, 128]
    for c in range(NCORES):
        m = dict(
            t0=t0dup,
            ego0sb=ego0sb[c],
            gidx=gidx[c],
            sgidx=sg_idx[c],
            sgdst=sg_dst[c],
            iota=iota,
            ident=ident,
        )
        if PSTREAM:
            pm = np.zeros((NC, 128, 128), np.float32)
            pm[qi.ravel(), pi.ravel(), relv[c].astype(np.int64).ravel()] = \
                valv[c].ravel()
            m["pmat"] = np.ascontiguousarray(
                pm.transpose(1, 0, 2).reshape(128, NC * 128)).astype(BF)
        else:
            m["relv"] = relv[c].astype(np.float32)
            m["valv"] = valv[c].astype(np.float32)
        m.update(wmats)
        in_maps.append(m)

    meta = dict(
        N=N, NP=NP, PPC=PPC, TPC=TPC, NRANGE=NRANGE, NC=NC,
        BPC=BPC, B3=B3, SC=SC, dins=dins, douts=douts, D0=D0,
        groups=groups, nchunk=nchunk, subk=subk, slot_of=slot_of,
        chunk_q=chunk_q, gidx_off=gidx_off, TOT16=TOT16,
        sg_cnt=sg_cnt, STOT16=STOT16, B=B,
    )
    return meta, in_maps


def _dma_gather_raw(gp, out_ap, in_ap, idxs_ap, num_idxs, elem_size, elem_step,
                    single_packet=False, queue_num=0):
    """Non-transpose HBM dma_gather allowing elem_size_bytes % 128 (the 256B
    restriction in bass.dma_gather only applies to transpose mode). Row
    stride (elem_step) must still be a 256B multiple."""
    from concourse import mybir as mb
    from concourse.ap_utils import ap_is_contiguous
    assert idxs_ap.dtype == mb.dt.int16
    assert in_ap.dtype == out_ap.dtype
    assert ap_is_contiguous(in_ap.ap[1:])
    assert ap_is_contiguous(out_ap.ap[1:])
    assert ap_is_contiguous(idxs_ap.ap[1:])
    assert in_ap.ap[0][0] == elem_step
    stride_bytes = elem_step * mb.dt.size(in_ap.dtype)
    assert stride_bytes % 256 == 0
    _in_ap = gp.lower_ap_dma(in_ap, for_custom_bir_dma=True)
    _idxs_ap = gp.lower_ap(idxs_ap)
    _out_ap = gp.lower_ap(out_ap)
    return gp.add_instruction(
        mb.InstDMAGatherAnt(
            name=gp.bass.get_next_instruction_name(),
            ins=[*_in_ap, _idxs_ap,
                 gp.lower_val_access(gp.to_reg(num_idxs))],
            outs=[_out_ap],
            transpose=False,
            num_idxs=num_idxs,
            elem_size=elem_size,
            stride_bytes_256=stride_bytes // 256,
            gen_mode=0,
            single_packet=single_packet,
            queue_num=queue_num,
            sbuf_tokens_per_rank=0,
            sbuf_free_dim_per_rank=0,
            sbuf_free_dim_pad_per_rank=0,
            sbuf_byte_offset=0,
        ))


def build_program(meta):
    f32, bf16, i16 = mybir.dt.float32, mybir.dt.bfloat16, mybir.dt.int16
    NP, PPC, TPC = meta["NP"], meta["PPC"], meta["TPC"]
    NRANGE, NC = meta["NRANGE"], meta["NC"]
    B3, SC = meta["B3"], meta["SC"]
    dins, douts = meta["dins"], meta["douts"]
    groups, nchunk = meta["groups"], meta["nchunk"]
    subk, slot_of, chunk_q = meta["subk"], meta["slot_of"], meta["chunk_q"]
    gidx_off, sg_cnt = meta["gidx_off"], meta["sg_cnt"]

    SP = os.environ.get("KGAT_SP", "0") == "1"
    NQ = int(os.environ.get("KGAT_NQ", "4"))
    SCRATCH = int(os.environ.get("KGAT_SCRATCH", "20480"))
    PSTREAM = os.environ.get("KGAT_PSTREAM", "1") == "1"
    PBLK = int(os.environ.get("KGAT_PBLK", "16"))
    S0STEP = int(os.environ.get("KGAT_S0", "16"))
    nc = bacc.Bacc(num_swdge_queues=NQ, dynamic_dma_scratch_size=SCRATCH)

    t0 = nc.dram_tensor("t0", [NP, 128], bf16, kind="ExternalInput")
    ego0sb_in = nc.dram_tensor("ego0sb", [128, TPC * 64], bf16, kind="ExternalInput")
    if PSTREAM:
        pmat_in = nc.dram_tensor("pmat", [128, NC * 128], bf16, kind="ExternalInput")
    else:
        relv_in = nc.dram_tensor("relv", [128, NC], f32, kind="ExternalInput")
        valv_in = nc.dram_tensor("valv", [128, NC], f32, kind="ExternalInput")
    gidx_in = nc.dram_tensor("gidx", [128, meta["TOT16"]], i16, kind="ExternalInput")
    sgidx_in = nc.dram_tensor("sgidx", [128, meta["STOT16"]], i16, kind="ExternalInput")
    sgdst_in = nc.dram_tensor("sgdst", [128, meta["STOT16"]], i16, kind="ExternalInput")
    iota_in = nc.dram_tensor("iota", [128, 128], bf16, kind="ExternalInput")
    ident_in = nc.dram_tensor("ident", [128, 128], bf16, kind="ExternalInput")
    w_in = {}
    for l in range(3):
        for nm in ("gc", "bi"):
            w_in[f"{nm}{l}"] = nc.dram_tensor(
                f"w_{nm}{l}", [dins[l] + 1, douts[l]], bf16, kind="ExternalInput")
    scores_out = nc.dram_tensor("scores", [128, 2 * B3], f32, kind="ExternalOutput")

    with tile.TileContext(nc) as tc:
        with (
            tc.tile_pool(name="const", bufs=1) as cpool,
            tc.tile_pool(name="big", bufs=1) as bigp,
            tc.tile_pool(name="gf", bufs=4) as gfp,
            tc.tile_pool(name="gb", bufs=3) as gbp,
            tc.tile_pool(name="pp", bufs=6) as ppool,
            tc.tile_pool(name="pm", bufs=3) as pmp,
            tc.tile_pool(name="dense", bufs=2) as dpool,
            tc.tile_pool(name="psA", bufs=4, space="PSUM") as psA,
            tc.tile_pool(name="psB", bufs=2, space="PSUM") as psB,
            tc.tile_pool(name="psC", bufs=2, space="PSUM") as psC,
            tc.tile_pool(name="dram", bufs=1, space="DRAM") as dram,
        ):
            # ---- constants ----
            iota_t = cpool.tile([128, 128], bf16, tag="iota")
            nc.sync.dma_start(iota_t[:], iota_in[:])
            ident_t = cpool.tile([128, 128], bf16, tag="ident")
            nc.sync.dma_start(ident_t[:], ident_in[:])
            w_t = {}
            for l in range(3):
                for nm in ("gc", "bi"):
                    w = cpool.tile([dins[l] + 1, douts[l]], bf16, tag=f"w{nm}{l}")
                    nc.sync.dma_start(w[:], w_in[f"{nm}{l}"][:])
                    w_t[f"{nm}{l}"] = w
            if not PSTREAM:
                relv_t = cpool.tile([128, NC], f32, tag="relv")
                nc.sync.dma_start(relv_t[:], relv_in[:])
                valv_t = cpool.tile([128, NC], f32, tag="valv")
                nc.sync.dma_start(valv_t[:], valv_in[:])

            # ---- persistent big tiles ----
            ego_sb = bigp.tile([128, TPC * 64], bf16, tag="ego")
            nc.sync.dma_start(ego_sb[:], ego0sb_in[:])
            side_sb = bigp.tile([128, TPC * 64], bf16, tag="side")
            inv_sb = [bigp.tile([128, TPC], f32, tag=f"inv{l}", name=f"inv{l}")
                      for l in range(3)]

            # ---- DRAM tables / pieces (bf16, rows [feat|feat] = 256B) ----
            tables = [t0]
            pieces = []
            for l in range(3):
                T = dram.tile([NP, 128], bf16, tag=f"T{l+1}", name=f"T{l+1}", addr_space="Shared")
                tables.append(T)
                pieces.append(dram.tile([PPC, 128], bf16, tag=f"piece{l+1}", name=f"piece{l+1}"))

            # ================= layers =================
            gq = [0]
            NLAYERS = int(os.environ.get("KGAT_LAYERS", "3"))
            SKIP_AG = os.environ.get("KGAT_SKIP_AG", "0") == "1"
            SKIP_SCORE = os.environ.get("KGAT_SKIP_SCORE", "0") == "1"
            SKIP_EDGE = os.environ.get("KGAT_SKIP_EDGE", "0") == "1"
            SKIP_ONEHOT = os.environ.get("KGAT_SKIP_ONEHOT", "0") == "1"
            SKIP_MM = os.environ.get("KGAT_SKIP_MM", "0") == "1"
            NBLK = -(-NC // PBLK)

            def pm_cols(b):
                return min(PBLK * 128, NC * 128 - b * PBLK * 128)

            for l in range(NLAYERS):
                din, dout = dins[l], douts[l]
                Tsrc = tables[l]

                pm_blocks = {}
                if PSTREAM and not (SKIP_EDGE or SKIP_ONEHOT):
                    for b in range(min(2, NBLK)):
                        pmt = pmp.tile([128, PBLK * 128], bf16, tag="pmb")
                        nc.sync.dma_start(
                            pmt[:, :pm_cols(b)],
                            pmat_in[:, b * PBLK * 128:b * PBLK * 128 + pm_cols(b)])
                        pm_blocks[b] = pmt

                for gi, ts in enumerate(groups):
                    gbase = ts[0]
                    Tg = len(ts)
                    if SKIP_EDGE:
                        nc.vector.memset(side_sb[:, gbase * 64:(gbase + Tg) * 64], 0.0)
                    # ---- r-major: gather range r, matmul its chunks into
                    # per-tile PSUM accumulators; gather r+1 overlaps ----
                    ps_t = {}
                    done_t = {t: 0 for t in ts}
                    tot_t = {t: int(nchunk[t].sum()) for t in ts}
                    if not SKIP_EDGE:
                        nbank = -(-Tg // 8)
                        banks = [psA.tile([128, 8, 64], f32, tag="psb",
                                          name=f"psb{gi}_{bi}")
                                 for bi in range(nbank)]
                        for ti, t in enumerate(ts):
                            if tot_t[t] > 0 and not SKIP_ONEHOT:
                                ps_t[t] = banks[ti // 8][:, ti % 8, :]
                            else:
                                nc.vector.memset(
                                    side_sb[:, t * 64:t * 64 + din], 0.0)
                    for r in range(NRANGE):
                        if SKIP_EDGE:
                            break
                        sk = subk[(gi, r)]
                        if sk == 0:
                            continue
                        wsz = min(WINDOW, NP - r * WINDOW)
                        off = gidx_off[(gi, r)]
                        gb = gbp.tile([128, sk, 64], bf16, tag="gb")
                        for s0 in range(0, sk, S0STEP):
                            skc = min(S0STEP, sk - s0)
                            idxt = gfp.tile([128, skc * 8], i16, tag="idx")
                            nc.sync.dma_start(
                                idxt[:], gidx_in[:, off + s0 * 8:off + (s0 + skc) * 8])
                            _dma_gather_raw(
                                nc.gpsimd,
                                out_ap=gb[:, s0:s0 + skc, :],
                                in_ap=Tsrc[r * WINDOW:r * WINDOW + wsz],
                                idxs_ap=idxt[:], num_idxs=skc * 128,
                                elem_size=64, elem_step=128,
                                single_packet=SP, queue_num=(gq[0] % NQ),
                            )
                            gq[0] += 1
                        if SKIP_ONEHOT:
                            continue
                        for t in ts:
                            for j in range(int(nchunk[t, r])):
                                qq = chunk_q[(t, r, j)]
                                s = slot_of[(t, r, j)]
                                if PSTREAM:
                                    b = qq // PBLK
                                    if b + 2 < NBLK and (b + 2) not in pm_blocks:
                                        pmt = pmp.tile(
                                            [128, PBLK * 128], bf16, tag="pmb")
                                        nc.sync.dma_start(
                                            pmt[:, :pm_cols(b + 2)],
                                            pmat_in[:, (b + 2) * PBLK * 128:
                                                    (b + 2) * PBLK * 128
                                                    + pm_cols(b + 2)])
                                        pm_blocks[b + 2] = pmt
                                    P_ap = pm_blocks[b][
                                        :, (qq - b * PBLK) * 128:
                                        (qq - b * PBLK) * 128 + 128]
                                else:
                                    P = ppool.tile([128, 128], bf16, tag="P")
                                    nc.vector.tensor_scalar(
                                        out=P[:], in0=iota_t[:],
                                        scalar1=relv_t[:, qq:qq + 1],
                                        scalar2=valv_t[:, qq:qq + 1],
                                        op0=ALU.is_equal, op1=ALU.mult,
                                    )
                                    P_ap = P[:]
                                if not SKIP_MM:
                                    nc.tensor.matmul(
                                        ps_t[t][:, :din], lhsT=P_ap,
                                        rhs=gb[:, s, :din],
                                        start=(done_t[t] == 0),
                                        stop=(done_t[t] == tot_t[t] - 1),
                                    )
                                done_t[t] += 1
                    for t in ts:
                        if SKIP_EDGE:
                            break
                        if t not in ps_t:
                            continue
                        if SKIP_MM:
                            nc.vector.memset(side_sb[:, t * 64:t * 64 + din], 0.0)
                        else:
                            nc.scalar.copy(out=side_sb[:, t * 64:t * 64 + din],
                                           in_=ps_t[t][:, :din])

                    # ---- dense phase for this group ----
                    sl3 = lambda big, w: big[:, gbase * 64:(gbase + Tg) * 64].rearrange(
                        "p (t d) -> p t d", d=64)[:, :, :w]
                    plus = dpool.tile([128, Tg, din + 1], bf16, tag="plus")
                    nc.vector.tensor_tensor(out=plus[:, :, :din], in0=sl3(side_sb, din),
                                            in1=sl3(ego_sb, din), op=ALU.add)
                    nc.vector.memset(plus[:, :, din], 1.0)
                    times = dpool.tile([128, Tg, din + 1], bf16, tag="times")
                    nc.vector.tensor_tensor(out=times[:, :, :din], in0=sl3(side_sb, din),
                                            in1=sl3(ego_sb, din), op=ALU.mult)
                    nc.vector.memset(times[:, :, din], 1.0)
                    bo = {}
                    for bname, src, wkey in (("p", plus, f"gc{l}"), ("b", times, f"bi{l}")):
                        out_b = dpool.tile([128, Tg * dout], f32, tag=f"bo{bname}")
                        for ti in range(Tg):
                            tp = psB.tile([din + 1, 128], bf16, tag="tp")
                            nc.tensor.transpose(tp[:], src[:, ti, :], ident_t[:])
                            xt = ppool.tile([din + 1, 128], bf16, tag="xt")
                            nc.vector.tensor_copy(out=xt[:], in_=tp[:])
                            mo = psC.tile([128, dout], f32, tag="mo")
                            nc.tensor.matmul(mo[:], lhsT=xt[:], rhs=w_t[wkey][:],
                                             start=True, stop=True)
                            ob = out_b[:, ti * dout:(ti + 1) * dout]
                            nc.scalar.activation(ob, mo[:], ACT.Copy, scale=0.01)
                            nc.vector.tensor_tensor(out=ob, in0=ob, in1=mo[:],
                                                    op=ALU.max)
                        bo[bname] = out_b
                    out_g = dpool.tile([128, Tg * 64], f32, tag="outg")
                    nc.vector.memset(out_g[:], 0.0)
                    og3 = out_g[:].rearrange("p (t d) -> p t d", d=64)[:, :, :dout]
                    nc.vector.tensor_tensor(out=og3, in0=bo["p"][:], in1=bo["b"][:],
                                            op=ALU.add)
                    # l2 norm factors
                    sq = dpool.tile([128, Tg * dout], f32, tag="sq")
                    nc.vector.tensor_tensor(out=sq[:], in0=og3, in1=og3, op=ALU.mult)
                    ssum = dpool.tile([128, Tg], f32, tag="ssum")
                    nc.vector.reduce_sum(
                        out=ssum[:], in_=sq[:].rearrange("p (t d) -> p t d", d=dout),
                        axis=mybir.AxisListType.X)
                    nrm = dpool.tile([128, Tg], f32, tag="nrm")
                    nc.scalar.activation(nrm[:], ssum[:], ACT.Sqrt)
                    nc.vector.tensor_scalar_max(out=nrm[:], in0=nrm[:], scalar1=1e-12)
                    nc.vector.reciprocal(inv_sb[l][:, gbase:gbase + Tg], nrm[:])
                    if l == 2:
                        og64 = out_g[:].rearrange("p (t d) -> p t d", d=64)
                        for i in range(3):
                            nc.vector.tensor_copy(
                                out=og64[:, :, 16 + i],
                                in_=inv_sb[i][:, gbase:gbase + Tg])
                    # update ego (bf16) and store duplicated bf16 piece
                    nc.vector.tensor_copy(
                        out=ego_sb[:, gbase * 64:(gbase + Tg) * 64], in_=out_g[:])
                    dup = dpool.tile([128, Tg, 128], bf16, tag="dup")
                    og64f = out_g[:].rearrange("p (t d) -> p t d", d=64)
                    nc.scalar.copy(out=dup[:, :, 0:64], in_=og64f)
                    nc.vector.tensor_copy(out=dup[:, :, 64:128], in_=og64f)
                    dst = pieces[l][:].rearrange("(p t) d -> p (t d)", p=128)
                    nc.sync.dma_start(
                        dst[:, gbase * 128:(gbase + Tg) * 128],
                        dup[:].rearrange("p t d -> p (t d)"))

                if not SKIP_AG:
                    nc.gpsimd.collective_compute(
                        "AllGather", ALU.bypass,
                        replica_groups=[list(range(NCORES))],
                        ins=[pieces[l].opt()], outs=[tables[l + 1].opt()],
                    )

            # ================= scoring =================
            if SKIP_SCORE:
                zz = dpool.tile([128, 2 * B3], f32, tag="zz")
                nc.vector.memset(zz[:], 0.0)
                nc.sync.dma_start(scores_out[:], zz[:])
            stage = []
            peer = None
            for ti in range(4) if not SKIP_SCORE else []:
                own = bigp.tile([128, SC, 128], bf16, tag=f"stown{ti}")
                if peer is None:
                    peer = bigp.tile([128, SC, 128], bf16, tag="stpeer")
                    nc.vector.memset(peer[:], 0.0)
                nc.vector.memset(own[:], 0.0)
                stage.append((own, peer))
            soff = 0
            for r in range(NRANGE) if not SKIP_SCORE else []:
                tot = sg_cnt[r]
                gi_t = gfp.tile([128, tot // 16], i16, tag="sgi")
                nc.sync.dma_start(gi_t[:], sgidx_in[:, soff:soff + tot // 16])
                gd_t = gfp.tile([128, tot // 16], i16, tag="sgd")
                nc.sync.dma_start(gd_t[:], sgdst_in[:, soff:soff + tot // 16])
                soff += tot // 16
                wsz = min(WINDOW, NP - r * WINDOW)
                for ti in range(4):
                    gf = gfp.tile([128, tot // 128, 128], bf16, tag="sgf")
                    nc.gpsimd.dma_gather(
                        out_ap=gf[:], in_ap=tables[ti][r * WINDOW:r * WINDOW + wsz],
                        idxs_ap=gi_t[:], num_idxs=tot, num_idxs_reg=tot,
                        elem_size=128, single_packet=SP,
                    )
                    own, peer = stage[ti]
                    nc.gpsimd.dma_scatter_add(
                        out_ap=own[:], in_ap=gf[:], idxs_ap=gd_t[:],
                        num_idxs=tot, num_idxs_reg=tot, elem_size=128,
                        sbuf_tokens_per_rank=128, parity_reg=0,
                        out_ap_other=peer[:], single_packet=False,
                    )
            # dots
            dls = [meta["D0"]] + douts
            acc = {}
            if SKIP_SCORE:
                dls = []
            for which, o1 in (("pos", 128), ("neg", 256)) if not SKIP_SCORE else []:
                total = dpool.tile([128, B3], f32, tag=f"tot{which}")
                for ti in range(4):
                    own = stage[ti][0]
                    dl = dls[ti]
                    u = own[:].rearrange("p c d -> p (c d)").rearrange(
                        "p (j x) -> p j x", x=384)[:, :, 0:dl]
                    v = own[:].rearrange("p c d -> p (c d)").rearrange(
                        "p (j x) -> p j x", x=384)[:, :, o1:o1 + dl]
                    prod = dpool.tile([128, B3 * dl], f32, tag="prod")
                    nc.vector.tensor_tensor(out=prod[:], in0=u, in1=v, op=ALU.mult)
                    d = dpool.tile([128, B3], f32, tag=f"dot{ti}{which}")
                    nc.vector.reduce_sum(
                        out=d[:], in_=prod[:].rearrange("p (j d) -> p j d", d=dl),
                        axis=mybir.AxisListType.X)
                    acc[(ti, which)] = d
                own3 = stage[3][0]
                for ti in range(1, 4):
                    col = 16 + ti - 1
                    iu = own3[:].rearrange("p c d -> p (c d)").rearrange(
                        "p (j x) -> p j x", x=384)[:, :, col]
                    iv = own3[:].rearrange("p c d -> p (c d)").rearrange(
                        "p (j x) -> p j x", x=384)[:, :, o1 + col]
                    d = acc[(ti, which)]
                    nc.vector.tensor_tensor(out=d[:], in0=d[:], in1=iu, op=ALU.mult)
                    nc.vector.tensor_tensor(out=d[:], in0=d[:], in1=iv, op=ALU.mult)
                nc.vector.tensor_tensor(out=total[:], in0=acc[(0, which)][:],
                                        in1=acc[(1, which)][:], op=ALU.add)
                nc.vector.tensor_tensor(out=total[:], in0=total[:],
                                        in1=acc[(2, which)][:], op=ALU.add)
                nc.vector.tensor_tensor(out=total[:], in0=total[:],
                                        in1=acc[(3, which)][:], op=ALU.add)
                acc[which] = total
            if not SKIP_SCORE:
                outt = dpool.tile([128, 2 * B3], f32, tag="outt")
                nc.vector.tensor_copy(out=outt[:, :B3], in_=acc["pos"][:])
                nc.vector.tensor_copy(out=outt[:, B3:], in_=acc["neg"][:])
                nc.sync.dma_start(scores_out[:], outt[:])

    nc.compile()
    return nc


def kernel(**inputs):
    meta, in_maps = build_host_data(inputs)
    nc = build_program(meta)
    trace = os.environ.get("KGAT_TRACE", "0") == "1"
    rr = run_bass_kernel_spmd(nc, in_maps, list(range(NCORES)), trace=trace)
    if trace and rr.exec_time_ns is not None:
        print(f"HW exec time: {rr.exec_time_ns} ns")
    if trace and rr.profile_json is not None:
        import json
        with open("/tmp/kgat_profile.json", "w") as f:
            json.dump(rr.profile_json, f)
    res = rr.results
    B3, BPC, B = meta["B3"], meta["BPC"], meta["B"]
    out = np.zeros((B, 2), np.float32)
    for c in range(NCORES):
        sc = res[c]["scores"]                       # [128, 2*B3]
        pos = sc[:, :B3]                            # [128, B3] (partition, jj)
        neg = sc[:, B3:]
        b = np.arange(BPC)
        out[c * BPC + b, 0] = pos[b % 128, b // 128]
        out[c * BPC + b, 1] = neg[b % 128, b // 128]
    return out

